# revision 28
# baseline (speedup 1.0000x reference)
"""Trainium2 raw-Bass kernel for nn_BatchDropTop (topk row masking).

Reference math: per sample b, act = sum_c x[b,c,:,:]^2 -> [H,W]; L2-normalize
over flattened (H,W) (positive per-sample scale -- order-preserving, skipped);
row score = max_w act -> [H]; zero the rh=8 rows with the largest score;
out = x * row_mask.

fp16 I/O (host casts): rel-err gate is 2e-2; selection was validated safe with
fp16 inputs + fp32 squares + fp32 accumulation (>=5.4e-6 relative margin on
all 64 samples).  fp16 squares are NOT safe; xsq stays fp32.

RAW Bass (no TileContext), manual semaphores.  Trace-driven structure:

  * The NEFF epilogue (walrus-emitted) makes EVERY engine (a) join a
    sem-2 arrival barrier and then (b) serially wait for every semaphore
    in its fixed ~51-sem hardware window to be 0 (Tensor's chain alone is
    ~55 x 115ns = 6.3us; the sweep also gates on the DMA-bookkeeping sems,
    i.e. it drains the queues).  Consequences engineered for here:
      - barrier-less block end (each engine branches to the end bb) so
        early-finishing engines reach the arrival barrier immediately;
      - all bass sems sit in SYNC's sweep window (207-255, the fastest
        chain at ~23ns/wait);
      - store-completion updates are +0 (nothing to wait on or clear), so
        the teardown does not wait out the final stores' wire time -- the
        epilogue sweep covers it;
      - sync/scalar post a 1-cycle "done" inc after their last trigger so
        the teardown can prove they passed their waits;
      - gpsimd (nothing else to do) waits semY==16 (DVE's last update --
        transitively the final value of EVERY sem), semSYD, semSCD, then
        dma_reset + sem_clear of the one contiguous sem range.
  * gpsimd runs NOTHING in the pipeline (its partition_broadcast measured
    3.7us/sample here vs 0.9us under Tile -- DMA-engine contention).  The
    mask broadcast maskhw[1,·] -> [P,·] is a PE ones-matmul (K=1, fp16
    single-pass) into PSUM, converted fp32->fp16 PSUM->SBUF by ACT
    (ScalarE sits closest to PSUM; ACT has ~2.5us/sample of slack).
  * Samples are processed in GROUPS [(0,),(1,),(2,3),(4,5),(6,7)]: the
    fill samples run alone for latency, later pairs share one DVE
    instruction per stage (fp32 fold fixed cost ~150ns/op plus each
    standalone sem-wait ~170ns of DVE queue time -- pairing halves both).

Dataflow per core (8 samples; per sample x is [P=128, KC=16, HW=192] f16,
partition p holds channels 16p..16p+15):
  loads:   s0 in fold-pair-aligned quarters (q0,q1 ring A / q2,q3 ring B
           so ACT can chase them), s1..s7 full tile on ring A (sync).
           Every load has a DEDICATED completion sem -- no cross-queue
           ordering assumptions.
  ACT:     square f16 -> f32 (one ACTIVATE per sample), m16 PSUM->SBUF
           copies per group, ring B triggers.
  DVE:     (pacer) L1/L2 contiguous fp32 folds per group; rowmax (PSUM),
           MAX8 top8 (per sample), maskhw compare per group; y = x*m16 IN
           PLACE on the x tile in two halves per sample (fp16 2x mode --
           a full-sample multiply loses it).  Software pipelined with
           stage skew: fold[g] | rowmax/max8/mask[g-1] | mults[g-2].
  PE:      four accumulating N=192 fp32 ones-matmuls per sample -> act
           [1, npair*192] PSUM + one mask-broadcast matmul per group.
  stores:  straight from the x tile (in-place mult => no y tiles/WAR).
           s0..s6 full on ring A; s7 in halves on ring B to parallelize
           the end drain across both rings.

The race model does not credit same-engine program order for data
visibility (and HW agrees: removing the same-engine waits broke real-HW
results) -- semDVE is the DVE self-clock; release points inc it, a wait at
value k implies everything program-order-before the k-th release.

Measured facts carried over (do not regress):
  - DVE fp32 tensor_tensor 1x ((N+151)/0.96ns); fp16 TT 2x_1P; strided
    tensor_reduce ~3x slower than contiguous TT folds.
  - fp16 anywhere in the fold tree flips the selection on this input set.
"""

import sys

import numpy as np

for _p in ("/opt/trn_rl_repo", "/root/.axon_site/_ro/trn_rl_repo"):
    if _p not in sys.path:
        sys.path.append(_p)

B, C, H, W = 64, 2048, 24, 8
N_CORES = 8
BS = B // N_CORES  # samples per core
P = 128            # SBUF partitions
KC = C // P        # channel chunks per sample (16)
KH = KC // 2       # 8
KQ = KC // 4       # 4
HW = H * W         # 192
RH = 8             # rows to drop == round(0.33 * 24)

# Sample groups: fill samples alone (pipeline latency), later pairs share
# DVE instructions.
GROUPS = [(0,), (1,), (2, 3), (4, 5), (6, 7)]
NG = len(GROUPS)

# First sem number for this kernel's sems: inside SYNC's NEFF-epilogue
# sweep window (207-255) -- see module docstring.
SEM_BASE = 210

_cache = {}


def _build_nc(tail="fast"):
    """tail="fast": barrier-less block end + minimal teardown (production).
    tail="barrier": standard Block exit (drains + all-engine barrier) +
    post-block clears -- structurally what the CoreSim race detector fully
    validates; the pipeline emission is IDENTICAL, so validating it
    validates the pipeline."""
    from contextlib import ExitStack, contextmanager

    from concourse import bacc, bass, mybir
    from concourse.bass import compact_to_ranges

    f32 = mybir.dt.float32
    f16 = mybir.dt.float16
    ADD = mybir.AluOpType.add
    MULT = mybir.AluOpType.mult

    class _NoBarrierBlock(bass.BassBlock):
        """BassBlock whose exit wires the end bb but emits NO all-engine
        barrier: each engine falls straight into the NEFF epilogue's own
        arrival barrier instead of idling behind a bass one too."""

        def __exit__(self, exc_type, exc_val, exc_tb):
            if exc_type is not None:
                return
            for engine, last_body in self.last_body.items():
                with self.bass.body(
                    last_body, parent=self.bass.cur_bb,
                    allow_existing_parent=True,
                ):
                    engine.br(self.end_bb)
            self.bass.switch_bb(self.end_bb)

    @contextmanager
    def no_barrier_block(nc, name):
        assert nc.cur_block is None
        with _NoBarrierBlock(nc, name) as blk:
            nc.cur_block = blk
            yield blk
        nc.cur_block = None

    nc = bacc.Bacc("TRN2", target_bir_lowering=False, debug=False,
                   num_devices=N_CORES,
                   detect_race_conditions=(tail == "barrier"))
    x_in = nc.dram_tensor("x", [BS, C, H, W], f16, kind="ExternalInput")
    y_out = nc.dram_tensor("out", [BS, C, H, W], f16, kind="ExternalOutput")

    es = ExitStack()
    with es:
        # --- SBUF / PSUM (double-buffered per GROUP) -----------------------
        xt = [es.enter_context(nc.sbuf_tensor(f"x{s}", [P, KC, HW], f16))
              for s in range(BS)]
        xsq = [es.enter_context(
                   nc.sbuf_tensor(f"xsq{i}", [P, 2, KC, HW], f32))
               for i in range(2)]
        t1 = [es.enter_context(nc.sbuf_tensor(f"t1_{i}", [P, 2, KH, HW],
                                              f32)) for i in range(2)]
        t2 = [es.enter_context(nc.sbuf_tensor(f"t2_{i}", [P, 2, KQ, HW],
                                              f32)) for i in range(2)]
        ones = es.enter_context(nc.sbuf_tensor("ones", [P, 1], f32))
        # fp16 so the K=1 broadcast matmul (fp16 x fp16 -> fp32 PSUM) is
        # single-pass; exact for 0/1 mask values.
        ones_row = es.enter_context(nc.sbuf_tensor("ones_row", [1, P], f16))
        rowmax = [es.enter_context(nc.sbuf_tensor(f"rm{i}", [1, 2, H], f32))
                  for i in range(2)]
        top8 = [es.enter_context(nc.sbuf_tensor(f"t8_{i}", [1, 2, RH], f32))
                for i in range(2)]
        maskhw = [es.enter_context(nc.sbuf_tensor(f"mh{i}", [1, 2, HW],
                                                  f16)) for i in range(2)]
        m16 = [es.enter_context(nc.sbuf_tensor(f"m16_{i}", [P, 2, HW], f16))
               for i in range(2)]
        act_ps = [es.enter_context(nc.psum_tensor(f"act{i}", [1, 2, HW],
                                                  f32)) for i in range(2)]
        bc_ps = [es.enter_context(nc.psum_tensor(f"bc{i}", [P, 2, HW], f32))
                 for i in range(2)]

        # --- semaphores (one contiguous range in SYNC's sweep window) ------
        semno = iter(range(SEM_BASE, 256))

        def sem(name):
            return es.enter_context(nc.semaphore(name, num=next(semno)))

        lq = [sem(f"lq{i}") for i in range(4)]      # s0 quarter loads
        lf = {s: sem(f"lf{s}") for s in range(1, BS)}  # full loads
        semSQ = sem("semSQ")      # ACT squares (4 for s0 quarters, 1/sample)
        semT2 = sem("semT2")      # DVE L2 done, 1/GROUP
        semACT = sem("semACT")    # PE act matmul group done, 1/SAMPLE
        semMH = sem("semMH")      # DVE maskhw done, 1/GROUP
        semBC = sem("semBC")      # PE mask-broadcast matmul done, 1/GROUP
        semM16 = sem("semM16")    # ACT m16 copy done, 1/GROUP
        semY = sem("semY")        # DVE mult halves, 2/SAMPLE
        semSTA = sem("semSTA")    # store completions (+0 updates)
        semONES = sem("semONES")  # ones memsets done
        semDVE = sem("semDVE")    # DVE self-clock
        semSYD = sem("semSYD")    # sync issued all triggers (passed waits)
        semSCD = sem("semSCD")    # scalar issued all triggers
        all_sems = (lq + list(lf.values())
                    + [semSQ, semT2, semACT, semMH, semBC, semM16, semY,
                       semSTA, semONES, semDVE, semSYD, semSCD])

        x_dram = [x_in[s].rearrange("(p k) h w -> p k (h w)", p=P)
                  for s in range(BS)]
        y_dram = [y_out[s].rearrange("(p k) h w -> p k (h w)", p=P)
                  for s in range(BS)]

        # semSQ value once sample s's square(s) are done (s0 = 4 quarters)
        def sq_val(s):
            return 4 + s

        # DVE clock bookkeeping: dve_clk[tag] = semDVE value after the
        # tagged release op.
        dve_clk = {"n": 0}

        def rel(inst, tag):
            inst.then_inc(semDVE, 1)
            dve_clk["n"] += 1
            dve_clk[tag] = dve_clk["n"]

        # Same-engine DVE data visibility is NOT given by program order
        # (measured on HW: removing these waits broke the results).  One
        # wait per true same-engine edge; acquired knowledge propagates
        # forward in program order.
        def dve_self_wait(vector, val):
            vector.wait_ge(semDVE, val)

        if tail == "fast":
            block_ctx = no_barrier_block(nc, "bdt")
        else:
            block_ctx = nc.Block("bdt", no_gpsimd_drain=True)
        with block_ctx as block:

            @block.sync
            def _(sync):
                # loads first (no deps): s0 quarters q0,q1 then s1..s7 full
                sync.dma_start(out=xt[0][:, 0 * KQ:1 * KQ, :],
                               in_=x_dram[0][:, 0 * KQ:1 * KQ, :]
                               ).then_inc(lq[0], 16)
                sync.dma_start(out=xt[0][:, 1 * KQ:2 * KQ, :],
                               in_=x_dram[0][:, 1 * KQ:2 * KQ, :]
                               ).then_inc(lq[1], 16)
                for s in range(1, BS):
                    sync.dma_start(out=xt[s][:], in_=x_dram[s][:]
                                   ).then_inc(lf[s], 16)
                # stores: x tiles hold y after the in-place multiply.  +0
                # completion updates (see module docstring).
                for s in range(BS - 1):
                    sync.wait_ge(semY, 2 * s + 2)
                    sync.dma_start(out=y_dram[s][:], in_=xt[s][:]
                                   ).then_inc(semSTA, 0, skip_validation=True)
                sync.sem_inc(semSYD, 1)

            @block.scalar
            def _(scalar):
                # ring B load triggers up-front: s0 quarters q2, q3.
                scalar.dma_start(out=xt[0][:, 2 * KQ:3 * KQ, :],
                                 in_=x_dram[0][:, 2 * KQ:3 * KQ, :]
                                 ).then_inc(lq[2], 16)
                scalar.dma_start(out=xt[0][:, 3 * KQ:4 * KQ, :],
                                 in_=x_dram[0][:, 3 * KQ:4 * KQ, :]
                                 ).then_inc(lq[3], 16)
                # sample 0 squared quarter-by-quarter in fold-pair order
                # (q0, q2 feed L1 piece A; q1, q3 feed piece B).
                for q in (0, 2, 1, 3):
                    scalar.wait_ge(lq[q], 16)
                    qs = slice(q * KQ, (q + 1) * KQ)
                    nc.scalar.square(xsq[0][:, 0, qs, :], xt[0][:, qs, :]
                                     ).then_inc(semSQ, 1)

                def sq_stage(g, r, s):
                    # xsq buffer WAR: DVE L2 of group g-2 consumed it.
                    scalar.wait_ge(lf[s], 16)
                    if g >= 2:
                        scalar.wait_ge(semT2, g - 1)
                    nc.scalar.square(xsq[g % 2][:, r], xt[s][:]
                                     ).then_inc(semSQ, 1)

                def cp_stage(g):
                    # m16 = fp16(bc_ps[g]): ScalarE is closest to PSUM.
                    # m16 buffer WAR: DVE mults of group g-2 done with it.
                    scalar.wait_ge(semBC, g + 1)
                    if g >= 2:
                        s_hi = GROUPS[g - 2][-1]
                        scalar.wait_ge(semY, 2 * s_hi + 2)
                    npair = len(GROUPS[g])
                    nc.scalar.copy(m16[g % 2][:, :npair],
                                   bc_ps[g % 2][:, :npair]
                                   ).then_inc(semM16, 1)

                # squares chase the loads; each group's copy is emitted
                # two groups behind (its bc matmul needs that group's
                # maskhw, which the DVE produces with one-slot skew).
                done_cp = 0
                for g, grp in enumerate(GROUPS):
                    for r, s in enumerate(grp):
                        if s != 0:
                            sq_stage(g, r, s)
                    if g >= 2:
                        cp_stage(done_cp)
                        done_cp += 1
                while done_cp < NG:
                    cp_stage(done_cp)
                    done_cp += 1

                # ring B end-drain: s7 stored in halves as its mults land.
                s = BS - 1
                for half in range(2):
                    ksl = slice(half * KH, (half + 1) * KH)
                    scalar.wait_ge(semY, 2 * s + 1 + half)
                    scalar.dma_start(out=y_dram[s][:, ksl, :],
                                     in_=xt[s][:, ksl, :]
                                     ).then_inc(semSTA, 0,
                                                skip_validation=True)
                scalar.sem_inc(semSCD, 1)

            @block.vector
            def _(vector):
                nc.vector.memset(ones[:], 1.0)
                nc.vector.memset(ones_row[:], 1.0).then_inc(semONES, 1)

                def l_stage(g):
                    grp = GROUPS[g]
                    npair = len(grp)
                    xq, tt1, tt2 = xsq[g % 2], t1[g % 2], t2[g % 2]
                    # t1 buffer WAR vs L2[g-2] read: L1[g-1] released after
                    # L2[g-2] in program order.
                    if g >= 2:
                        dve_self_wait(vector, dve_clk[f"L1_{g - 1}"])
                    if g == 0:
                        # chase the quarter squares (q0+q2 then q1+q3)
                        vector.wait_ge(semSQ, 2)
                        nc.vector.tensor_tensor(
                            tt1[:, 0, 0:KQ, :], xq[:, 0, 0:KQ, :],
                            xq[:, 0, 2 * KQ:3 * KQ, :], op=ADD)
                        vector.wait_ge(semSQ, 4)
                        rel(nc.vector.tensor_tensor(
                            tt1[:, 0, KQ:, :], xq[:, 0, KQ:2 * KQ, :],
                            xq[:, 0, 3 * KQ:, :], op=ADD), f"L1_{g}")
                    else:
                        vector.wait_ge(semSQ, sq_val(grp[-1]))
                        rel(nc.vector.tensor_tensor(
                            tt1[:, :npair], xq[:, :npair, :KH, :],
                            xq[:, :npair, KH:, :], op=ADD), f"L1_{g}")
                    # L2 fold; t2 buffer WAR: PE done with group g-2
                    if g >= 2:
                        vector.wait_ge(semACT, GROUPS[g - 2][-1] + 1)
                    dve_self_wait(vector, dve_clk[f"L1_{g}"])
                    nc.vector.tensor_tensor(
                        tt2[:, :npair], tt1[:, :npair, :KQ, :],
                        tt1[:, :npair, KQ:, :], op=ADD
                    ).then_inc(semT2, 1)

                def r_stage(g):
                    grp = GROUPS[g]
                    npair = len(grp)
                    rm, t8, mh = rowmax[g % 2], top8[g % 2], maskhw[g % 2]
                    vector.wait_ge(semACT, grp[-1] + 1)
                    # rm/t8 buffer WAR vs maskhw[g-2] reads: rowmax[g-1]
                    # released after maskhw[g-2] in program order.
                    if g >= 2:
                        dve_self_wait(vector, dve_clk[f"RM_{g - 1}"])
                    rel(nc.vector.tensor_reduce(
                        rm[:, :npair],
                        act_ps[g % 2][:, :npair].rearrange(
                            "p n (h w) -> p n h w", h=H),
                        axis=mybir.AxisListType.X,
                        op=mybir.AluOpType.max), f"RM_{g}")
                    dve_self_wait(vector, dve_clk[f"RM_{g}"])
                    for r in range(npair):
                        rel(nc.vector.max(t8[:, r], rm[:, r]), f"M8_{g}")
                    # maskhw buffer WAR: PE bcast of g-2 done reading it
                    if g >= 2:
                        vector.wait_ge(semBC, g - 1)
                    dve_self_wait(vector, dve_clk[f"M8_{g}"])
                    # mask = (rowmax < per-sample 8th-largest), fp16 0/1
                    nc.vector.tensor_tensor(
                        mh[:, :npair].rearrange("p n (h w) -> p n h w",
                                                h=H),
                        rm[:, :npair].unsqueeze(3).broadcast_to(
                            [1, npair, H, W]),
                        t8[:, :npair, RH - 1:RH].broadcast_to(
                            [1, npair, H]).unsqueeze(3).broadcast_to(
                            [1, npair, H, W]),
                        op=mybir.AluOpType.is_lt,
                    ).then_inc(semMH, 1)

                def m_stage(g):
                    # y = x * m16 in place, two halves per sample (fp16 2x
                    # mode).  Upstream deps arrive transitively through
                    # semM16's acquire chain.
                    vector.wait_ge(semM16, g + 1)
                    for r, s in enumerate(GROUPS[g]):
                        mb = m16[g % 2][:, r].unsqueeze(1).broadcast_to(
                            [P, KH, HW])
                        for half in range(2):
                            ksl = slice(half * KH, (half + 1) * KH)
                            nc.vector.tensor_tensor(
                                xt[s][:, ksl, :], xt[s][:, ksl, :], mb,
                                op=MULT).then_inc(semY, 1)

                for slot in range(NG + 2):
                    if slot < NG:
                        l_stage(slot)
                    if 1 <= slot <= NG:
                        r_stage(slot - 1)
                    if slot >= 2:
                        m_stage(slot - 2)

            @block.tensor
            def _(tensor):
                tensor.wait_ge(semONES, 1)

                def act_mm(g):
                    grp = GROUPS[g]
                    tensor.wait_ge(semT2, g + 1)
                    if g >= 2:
                        # act_ps WAR: DVE rowmax of g-2 consumed it
                        tensor.wait_ge(semDVE, dve_clk[f"RM_{g - 2}"])
                    for r, s in enumerate(grp):
                        for j in range(KQ):
                            mm = nc.tensor.matmul(
                                act_ps[g % 2][:, r], ones[:],
                                t2[g % 2][:, r, j, :],
                                start=(j == 0), stop=(j == KQ - 1))
                        mm.then_inc(semACT, 1)

                def bc_mm(g):
                    # broadcast maskhw[1,npair*HW] to all partitions:
                    # K=1 fp16 matmul -> bc_ps [P, npair*HW] fp32.
                    npair = len(GROUPS[g])
                    tensor.wait_ge(semMH, g + 1)
                    if g >= 2:
                        # bc_ps WAR: ACT copy of g-2 consumed it
                        tensor.wait_ge(semM16, g - 1)
                    nc.tensor.matmul(bc_ps[g % 2][:, :npair], ones_row[:],
                                     maskhw[g % 2][:, :npair],
                                     start=True, stop=True
                                     ).then_inc(semBC, 1)

                for g in range(NG):
                    act_mm(g)
                    if g >= 1:
                        bc_mm(g - 1)
                bc_mm(NG - 1)

            @block.gpsimd
            def _(gpsimd):
                # Teardown only.  semY==16 is posted by DVE's last op,
                # which sits after every DVE wait -- transitively it proves
                # EVERY sem reached its final value.  semSYD/semSCD prove
                # sync and scalar issued their last triggers, i.e. passed
                # all their waits.  Store completions post +0: nothing to
                # wait for or clear.  Then zero the sems for the next NEFF
                # execution; the epilogue sweep's ==0 waits gate on this.
                gpsimd.wait_ge(semY, 2 * BS)
                gpsimd.wait_ge(semSYD, 1)
                gpsimd.wait_ge(semSCD, 1)
                if tail == "fast":
                    for rng in compact_to_ranges(sorted(s_.num
                                                        for s_ in all_sems)):
                        gpsimd.dma_reset(rng)
                        gpsimd.sem_clear(rng)

        if tail == "barrier":
            # race-detector-approved: Block exit emitted drains + an
            # all-engine barrier; clear after it.
            for rng in compact_to_ranges(sorted(s_.num for s_ in all_sems)):
                nc.gpsimd.dma_reset(rng)
                nc.gpsimd.sem_clear(rng)

    nc.compile()
    return nc


def get_nc():
    if "nc" not in _cache:
        _cache["nc"] = _build_nc()
    return _cache["nc"]


def kernel(x):
    from concourse.bass_utils import run_bass_kernel_spmd

    x = np.ascontiguousarray(np.asarray(x, dtype=np.float16))
    assert x.shape == (B, C, H, W), x.shape
    nc = get_nc()
    in_maps = [{"x": x[i * BS:(i + 1) * BS]} for i in range(N_CORES)]
    res = run_bass_kernel_spmd(nc, in_maps, list(range(N_CORES)))
    return np.concatenate(
        [res.results[i]["out"] for i in range(N_CORES)], axis=0
    ).astype(np.float32)


# revision 29
# speedup vs baseline: 1.0176x; 1.0176x over previous
"""Trainium2 raw-Bass kernel for nn_BatchDropTop (topk row masking).

Reference math: per sample b, act = sum_c x[b,c,:,:]^2 -> [H,W]; L2-normalize
over flattened (H,W) (positive per-sample scale -- order-preserving, skipped);
row score = max_w act -> [H]; zero the rh=8 rows with the largest score;
out = x * row_mask.

fp16 I/O (host casts): rel-err gate is 2e-2; selection was validated safe with
fp16 inputs + fp32 squares + fp32 accumulation (>=5.4e-6 relative margin on
all 64 samples).  fp16 squares are NOT safe; xsq stays fp32.

RAW Bass (no TileContext), manual semaphores.  Trace-driven structure:

  * The NEFF epilogue (walrus-emitted) makes EVERY engine (a) join a
    sem-2 arrival barrier and then (b) serially wait for every semaphore
    in its fixed ~51-sem hardware window to be 0 (Tensor's chain alone is
    ~55 x 115ns = 6.3us; the sweep also gates on the DMA-bookkeeping sems,
    i.e. it drains the queues).  Consequences engineered for here:
      - barrier-less block end (each engine branches to the end bb) so
        early-finishing engines reach the arrival barrier immediately;
      - all bass sems sit in SYNC's sweep window (207-255, the fastest
        chain at ~23ns/wait);
      - store-completion updates are +0 (nothing to wait on or clear), so
        the teardown does not wait out the final stores' wire time -- the
        epilogue sweep covers it;
      - sync/scalar post a 1-cycle "done" inc after their last trigger so
        the teardown can prove they passed their waits;
      - gpsimd (nothing else to do) waits semY==16 (DVE's last update --
        transitively the final value of EVERY sem), semSYD, semSCD, then
        dma_reset + sem_clear of the one contiguous sem range.
  * gpsimd runs NOTHING in the pipeline (its partition_broadcast measured
    3.7us/sample here vs 0.9us under Tile -- DMA-engine contention).  The
    mask broadcast maskhw[1,·] -> [P,·] is a PE ones-matmul (K=1, fp16
    single-pass) into PSUM, converted fp32->fp16 PSUM->SBUF by ACT
    (ScalarE sits closest to PSUM; ACT has ~2.5us/sample of slack).
  * Samples are processed in GROUPS [(0,),(1,),(2,3),(4,5),(6,7)]: the
    fill samples run alone for latency, later pairs share one DVE
    instruction per stage (fp32 fold fixed cost ~150ns/op plus each
    standalone sem-wait ~170ns of DVE queue time -- pairing halves both).

Dataflow per core (8 samples; per sample x is [P=128, KC=16, HW=192] f16,
partition p holds channels 16p..16p+15):
  loads:   s0 in fold-pair-aligned quarters (q0,q1 ring A / q2,q3 ring B
           so ACT can chase them), s1..s7 full tile on ring A (sync).
           Every load has a DEDICATED completion sem -- no cross-queue
           ordering assumptions.
  ACT:     square f16 -> f32 (one ACTIVATE per sample), m16 PSUM->SBUF
           copies per group, ring B triggers.
  DVE:     (pacer) L1/L2 contiguous fp32 folds per group; rowmax (PSUM),
           MAX8 top8 (per sample), maskhw compare per group; y = x*m16 IN
           PLACE on the x tile in two halves per sample (fp16 2x mode --
           a full-sample multiply loses it).  Software pipelined with
           stage skew: fold[g] | rowmax/max8/mask[g-1] | mults[g-2].
  PE:      four accumulating N=192 fp32 ones-matmuls per sample -> act
           [1, npair*192] PSUM + one mask-broadcast matmul per group.
  stores:  straight from the x tile (in-place mult => no y tiles/WAR).
           s0..s6 full on ring A; s7 in halves on ring B to parallelize
           the end drain across both rings.

The race model does not credit same-engine program order for data
visibility (and HW agrees: removing the same-engine waits broke real-HW
results) -- semDVE is the DVE self-clock; release points inc it, a wait at
value k implies everything program-order-before the k-th release.

Measured facts carried over (do not regress):
  - DVE fp32 tensor_tensor 1x ((N+151)/0.96ns); fp16 TT 2x_1P; strided
    tensor_reduce ~3x slower than contiguous TT folds.
  - fp16 anywhere in the fold tree flips the selection on this input set.
"""

import sys

import numpy as np

for _p in ("/opt/trn_rl_repo", "/root/.axon_site/_ro/trn_rl_repo"):
    if _p not in sys.path:
        sys.path.append(_p)

B, C, H, W = 64, 2048, 24, 8
N_CORES = 8
BS = B // N_CORES  # samples per core
P = 128            # SBUF partitions
KC = C // P        # channel chunks per sample (16)
KH = KC // 2       # 8
KQ = KC // 4       # 4
HW = H * W         # 192
RH = 8             # rows to drop == round(0.33 * 24)

# Sample groups: fill samples alone (pipeline latency), later pairs share
# DVE instructions.
GROUPS = [(0,), (1,), (2, 3), (4, 5), (6, 7)]
NG = len(GROUPS)

# First sem number for this kernel's sems: inside SYNC's NEFF-epilogue
# sweep window (207-255) -- see module docstring.
SEM_BASE = 210

_cache = {}


def _build_nc(tail="fast"):
    """tail="fast": barrier-less block end + minimal teardown (production).
    tail="barrier": standard Block exit (drains + all-engine barrier) +
    post-block clears -- structurally what the CoreSim race detector fully
    validates; the pipeline emission is IDENTICAL, so validating it
    validates the pipeline."""
    from contextlib import ExitStack, contextmanager

    from concourse import bacc, bass, mybir
    from concourse.bass import compact_to_ranges

    f32 = mybir.dt.float32
    f16 = mybir.dt.float16
    ADD = mybir.AluOpType.add
    MULT = mybir.AluOpType.mult

    class _NoBarrierBlock(bass.BassBlock):
        """BassBlock whose exit wires the end bb but emits NO all-engine
        barrier: each engine falls straight into the NEFF epilogue's own
        arrival barrier instead of idling behind a bass one too."""

        def __exit__(self, exc_type, exc_val, exc_tb):
            if exc_type is not None:
                return
            for engine, last_body in self.last_body.items():
                with self.bass.body(
                    last_body, parent=self.bass.cur_bb,
                    allow_existing_parent=True,
                ):
                    engine.br(self.end_bb)
            self.bass.switch_bb(self.end_bb)

    @contextmanager
    def no_barrier_block(nc, name):
        assert nc.cur_block is None
        with _NoBarrierBlock(nc, name) as blk:
            nc.cur_block = blk
            yield blk
        nc.cur_block = None

    nc = bacc.Bacc("TRN2", target_bir_lowering=False, debug=False,
                   num_devices=N_CORES,
                   detect_race_conditions=(tail == "barrier"))
    x_in = nc.dram_tensor("x", [BS, C, H, W], f16, kind="ExternalInput")
    y_out = nc.dram_tensor("out", [BS, C, H, W], f16, kind="ExternalOutput")

    es = ExitStack()
    with es:
        # --- SBUF / PSUM (double-buffered per GROUP) -----------------------
        xt = [es.enter_context(nc.sbuf_tensor(f"x{s}", [P, KC, HW], f16))
              for s in range(BS)]
        xsq = [es.enter_context(
                   nc.sbuf_tensor(f"xsq{i}", [P, 2, KC, HW], f32))
               for i in range(2)]
        t1 = [es.enter_context(nc.sbuf_tensor(f"t1_{i}", [P, 2, KH, HW],
                                              f32)) for i in range(2)]
        t2 = [es.enter_context(nc.sbuf_tensor(f"t2_{i}", [P, 2, KQ, HW],
                                              f32)) for i in range(2)]
        ones = es.enter_context(nc.sbuf_tensor("ones", [P, 1], f32))
        # fp16 so the K=1 broadcast matmul (fp16 x fp16 -> fp32 PSUM) is
        # single-pass; exact for 0/1 mask values.
        ones_row = es.enter_context(nc.sbuf_tensor("ones_row", [1, P], f16))
        rowmax = [es.enter_context(nc.sbuf_tensor(f"rm{i}", [1, 2, H], f32))
                  for i in range(2)]
        top8 = [es.enter_context(nc.sbuf_tensor(f"t8_{i}", [1, 2, RH], f32))
                for i in range(2)]
        maskhw = [es.enter_context(nc.sbuf_tensor(f"mh{i}", [1, 2, HW],
                                                  f16)) for i in range(2)]
        m16 = [es.enter_context(nc.sbuf_tensor(f"m16_{i}", [P, 2, HW], f16))
               for i in range(2)]
        act_ps = [es.enter_context(nc.psum_tensor(f"act{i}", [1, 2, HW],
                                                  f32)) for i in range(2)]
        bc_ps = [es.enter_context(nc.psum_tensor(f"bc{i}", [P, 2, HW], f32))
                 for i in range(2)]

        # --- semaphores (one contiguous range in SYNC's sweep window) ------
        semno = iter(range(SEM_BASE, 256))

        def sem(name):
            return es.enter_context(nc.semaphore(name, num=next(semno)))

        lq = [sem(f"lq{i}") for i in range(4)]      # s0 quarter loads
        lf = {s: sem(f"lf{s}") for s in range(1, BS)}  # full loads
        semSQ = sem("semSQ")      # ACT squares (4 for s0 quarters, 1/sample)
        semT2 = sem("semT2")      # DVE L2 done, 1/GROUP
        semACT = sem("semACT")    # PE act matmul group done, 1/SAMPLE
        semMH = sem("semMH")      # DVE maskhw done, 1/GROUP
        semBC = sem("semBC")      # PE mask-broadcast matmul done, 1/GROUP
        semM16 = sem("semM16")    # ACT m16 copy done, 1/GROUP
        semY = sem("semY")        # DVE mult halves, 2/SAMPLE
        semSTA = sem("semSTA")    # store completions (+0 updates)
        semONES = sem("semONES")  # ones memsets done
        semDVE = sem("semDVE")    # DVE self-clock
        semSYD = sem("semSYD")    # sync issued all triggers (passed waits)
        semSCD = sem("semSCD")    # scalar issued all triggers
        all_sems = (lq + list(lf.values())
                    + [semSQ, semT2, semACT, semMH, semBC, semM16, semY,
                       semSTA, semONES, semDVE, semSYD, semSCD])

        x_dram = [x_in[s].rearrange("(p k) h w -> p k (h w)", p=P)
                  for s in range(BS)]
        y_dram = [y_out[s].rearrange("(p k) h w -> p k (h w)", p=P)
                  for s in range(BS)]

        # semSQ value once ACT's square(s) for sample s are done
        # (s0 = 4 quarters; s1 is squared by the DVE itself in its fill
        # gap, so ACT skips it).
        def sq_val(s):
            assert s != 1
            return 4 if s == 0 else 3 + s

        # DVE clock bookkeeping: dve_clk[tag] = semDVE value after the
        # tagged release op.
        dve_clk = {"n": 0}

        def rel(inst, tag):
            inst.then_inc(semDVE, 1)
            dve_clk["n"] += 1
            dve_clk[tag] = dve_clk["n"]

        # Same-engine DVE data visibility is NOT given by program order
        # (measured on HW: removing these waits broke the results).  One
        # wait per true same-engine edge; acquired knowledge propagates
        # forward in program order.
        def dve_self_wait(vector, val):
            vector.wait_ge(semDVE, val)

        if tail == "fast":
            block_ctx = no_barrier_block(nc, "bdt")
        else:
            block_ctx = nc.Block("bdt", no_gpsimd_drain=True)
        with block_ctx as block:

            @block.sync
            def _(sync):
                # loads first (no deps): s0 quarters q0,q1 then s1..s7 full
                sync.dma_start(out=xt[0][:, 0 * KQ:1 * KQ, :],
                               in_=x_dram[0][:, 0 * KQ:1 * KQ, :]
                               ).then_inc(lq[0], 16)
                sync.dma_start(out=xt[0][:, 1 * KQ:2 * KQ, :],
                               in_=x_dram[0][:, 1 * KQ:2 * KQ, :]
                               ).then_inc(lq[1], 16)
                for s in range(1, BS):
                    sync.dma_start(out=xt[s][:], in_=x_dram[s][:]
                                   ).then_inc(lf[s], 16)
                # stores: x tiles hold y after the in-place multiply.  +0
                # completion updates (see module docstring).  s5/s7 go on
                # ring B (scalar) so the end drain runs on both rings;
                # s6/s7 store in halves to start their wire earlier.
                for s in (0, 1, 2, 3, 4):
                    sync.wait_ge(semY, 2 * s + 2)
                    sync.dma_start(out=y_dram[s][:], in_=xt[s][:]
                                   ).then_inc(semSTA, 0, skip_validation=True)
                s = 6
                for half in range(2):
                    ksl = slice(half * KH, (half + 1) * KH)
                    sync.wait_ge(semY, 2 * s + 1 + half)
                    sync.dma_start(out=y_dram[s][:, ksl, :],
                                   in_=xt[s][:, ksl, :]
                                   ).then_inc(semSTA, 0, skip_validation=True)
                sync.sem_inc(semSYD, 1)

            @block.scalar
            def _(scalar):
                # ring B load triggers up-front: s0 quarters q2, q3.
                scalar.dma_start(out=xt[0][:, 2 * KQ:3 * KQ, :],
                                 in_=x_dram[0][:, 2 * KQ:3 * KQ, :]
                                 ).then_inc(lq[2], 16)
                scalar.dma_start(out=xt[0][:, 3 * KQ:4 * KQ, :],
                                 in_=x_dram[0][:, 3 * KQ:4 * KQ, :]
                                 ).then_inc(lq[3], 16)
                # sample 0 squared quarter-by-quarter in fold-pair order
                # (q0, q2 feed L1 piece A; q1, q3 feed piece B).
                for q in (0, 2, 1, 3):
                    scalar.wait_ge(lq[q], 16)
                    qs = slice(q * KQ, (q + 1) * KQ)
                    nc.scalar.square(xsq[0][:, 0, qs, :], xt[0][:, qs, :]
                                     ).then_inc(semSQ, 1)

                def sq_stage(g, r, s):
                    # xsq buffer WAR: DVE L2 of group g-2 consumed it.
                    scalar.wait_ge(lf[s], 16)
                    if g >= 2:
                        scalar.wait_ge(semT2, g - 1)
                    nc.scalar.square(xsq[g % 2][:, r], xt[s][:]
                                     ).then_inc(semSQ, 1)

                def cp_stage(g):
                    # m16 = fp16(bc_ps[g]): ScalarE is closest to PSUM.
                    # m16 buffer WAR: DVE mults of group g-2 done with it.
                    scalar.wait_ge(semBC, g + 1)
                    if g >= 2:
                        s_hi = GROUPS[g - 2][-1]
                        scalar.wait_ge(semY, 2 * s_hi + 2)
                    npair = len(GROUPS[g])
                    nc.scalar.copy(m16[g % 2][:, :npair],
                                   bc_ps[g % 2][:, :npair]
                                   ).then_inc(semM16, 1)

                # squares chase the loads; each group's copy is emitted
                # two groups behind (its bc matmul needs that group's
                # maskhw, which the DVE produces with one-slot skew).
                done_cp = 0
                for g, grp in enumerate(GROUPS):
                    for r, s in enumerate(grp):
                        if s not in (0, 1):
                            sq_stage(g, r, s)
                    if g >= 2:
                        cp_stage(done_cp)
                        done_cp += 1
                while done_cp < NG:
                    cp_stage(done_cp)
                    done_cp += 1

                # ring B end-drain: s5 full, then s7 in halves.
                s = 5
                scalar.wait_ge(semY, 2 * s + 2)
                scalar.dma_start(out=y_dram[s][:], in_=xt[s][:]
                                 ).then_inc(semSTA, 0, skip_validation=True)
                s = BS - 1
                for half in range(2):
                    ksl = slice(half * KH, (half + 1) * KH)
                    scalar.wait_ge(semY, 2 * s + 1 + half)
                    scalar.dma_start(out=y_dram[s][:, ksl, :],
                                     in_=xt[s][:, ksl, :]
                                     ).then_inc(semSTA, 0,
                                                skip_validation=True)
                scalar.sem_inc(semSCD, 1)

            @block.vector
            def _(vector):
                nc.vector.memset(ones[:], 1.0)
                nc.vector.memset(ones_row[:], 1.0).then_inc(semONES, 1)

                def l_stage(g):
                    grp = GROUPS[g]
                    npair = len(grp)
                    xq, tt1, tt2 = xsq[g % 2], t1[g % 2], t2[g % 2]
                    # t1 buffer WAR vs L2[g-2] read: L1[g-1] released after
                    # L2[g-2] in program order.
                    if g >= 2:
                        dve_self_wait(vector, dve_clk[f"L1_{g - 1}"])
                    if g == 0:
                        # chase the quarter squares (q0+q2 then q1+q3)
                        vector.wait_ge(semSQ, 2)
                        nc.vector.tensor_tensor(
                            tt1[:, 0, 0:KQ, :], xq[:, 0, 0:KQ, :],
                            xq[:, 0, 2 * KQ:3 * KQ, :], op=ADD)
                        vector.wait_ge(semSQ, 4)
                        rel(nc.vector.tensor_tensor(
                            tt1[:, 0, KQ:, :], xq[:, 0, KQ:2 * KQ, :],
                            xq[:, 0, 3 * KQ:, :], op=ADD), f"L1_{g}")
                    elif g == 1:
                        dve_self_wait(vector, dve_clk["SQ1"])
                        rel(nc.vector.tensor_tensor(
                            tt1[:, :npair], xq[:, :npair, :KH, :],
                            xq[:, :npair, KH:, :], op=ADD), f"L1_{g}")
                    else:
                        vector.wait_ge(semSQ, sq_val(grp[-1]))
                        rel(nc.vector.tensor_tensor(
                            tt1[:, :npair], xq[:, :npair, :KH, :],
                            xq[:, :npair, KH:, :], op=ADD), f"L1_{g}")
                    # L2 fold; t2 buffer WAR: PE done with group g-2
                    if g >= 2:
                        vector.wait_ge(semACT, GROUPS[g - 2][-1] + 1)
                    dve_self_wait(vector, dve_clk[f"L1_{g}"])
                    nc.vector.tensor_tensor(
                        tt2[:, :npair], tt1[:, :npair, :KQ, :],
                        tt1[:, :npair, KQ:, :], op=ADD
                    ).then_inc(semT2, 1)

                def r_stage(g):
                    grp = GROUPS[g]
                    npair = len(grp)
                    rm, t8, mh = rowmax[g % 2], top8[g % 2], maskhw[g % 2]
                    vector.wait_ge(semACT, grp[-1] + 1)
                    # rm/t8 buffer WAR vs maskhw[g-2] reads: rowmax[g-1]
                    # released after maskhw[g-2] in program order.
                    if g >= 2:
                        dve_self_wait(vector, dve_clk[f"RM_{g - 1}"])
                    rel(nc.vector.tensor_reduce(
                        rm[:, :npair],
                        act_ps[g % 2][:, :npair].rearrange(
                            "p n (h w) -> p n h w", h=H),
                        axis=mybir.AxisListType.X,
                        op=mybir.AluOpType.max), f"RM_{g}")
                    dve_self_wait(vector, dve_clk[f"RM_{g}"])
                    for r in range(npair):
                        rel(nc.vector.max(t8[:, r], rm[:, r]), f"M8_{g}")
                    # maskhw buffer WAR: PE bcast of g-2 done reading it
                    if g >= 2:
                        vector.wait_ge(semBC, g - 1)
                    dve_self_wait(vector, dve_clk[f"M8_{g}"])
                    # mask = (rowmax < per-sample 8th-largest), fp16 0/1
                    nc.vector.tensor_tensor(
                        mh[:, :npair].rearrange("p n (h w) -> p n h w",
                                                h=H),
                        rm[:, :npair].unsqueeze(3).broadcast_to(
                            [1, npair, H, W]),
                        t8[:, :npair, RH - 1:RH].broadcast_to(
                            [1, npair, H]).unsqueeze(3).broadcast_to(
                            [1, npair, H, W]),
                        op=mybir.AluOpType.is_lt,
                    ).then_inc(semMH, 1)

                def m_stage(g):
                    # y = x * m16 in place, two halves per sample (fp16 2x
                    # mode).  Upstream deps arrive transitively through
                    # semM16's acquire chain.
                    vector.wait_ge(semM16, g + 1)
                    for r, s in enumerate(GROUPS[g]):
                        mb = m16[g % 2][:, r].unsqueeze(1).broadcast_to(
                            [P, KH, HW])
                        for half in range(2):
                            ksl = slice(half * KH, (half + 1) * KH)
                            nc.vector.tensor_tensor(
                                xt[s][:, ksl, :], xt[s][:, ksl, :], mb,
                                op=MULT).then_inc(semY, 1)

                for slot in range(NG + 2):
                    if slot == 1:
                        # square s1 on the DVE itself: during the fill the
                        # DVE would otherwise idle waiting for ACT, which
                        # is the serial bottleneck early on.
                        vector.wait_ge(lf[1], 16)
                        rel(nc.vector.tensor_tensor(
                            xsq[1][:, 0], xt[1][:], xt[1][:], op=MULT),
                            "SQ1")
                    if slot < NG:
                        l_stage(slot)
                    if 1 <= slot <= NG:
                        r_stage(slot - 1)
                    if slot >= 2:
                        m_stage(slot - 2)

            @block.tensor
            def _(tensor):
                tensor.wait_ge(semONES, 1)

                def act_mm(g):
                    grp = GROUPS[g]
                    tensor.wait_ge(semT2, g + 1)
                    if g >= 2:
                        # act_ps WAR: DVE rowmax of g-2 consumed it
                        tensor.wait_ge(semDVE, dve_clk[f"RM_{g - 2}"])
                    for r, s in enumerate(grp):
                        for j in range(KQ):
                            mm = nc.tensor.matmul(
                                act_ps[g % 2][:, r], ones[:],
                                t2[g % 2][:, r, j, :],
                                start=(j == 0), stop=(j == KQ - 1))
                        mm.then_inc(semACT, 1)

                def bc_mm(g):
                    # broadcast maskhw[1,npair*HW] to all partitions:
                    # K=1 fp16 matmul -> bc_ps [P, npair*HW] fp32.
                    npair = len(GROUPS[g])
                    tensor.wait_ge(semMH, g + 1)
                    if g >= 2:
                        # bc_ps WAR: ACT copy of g-2 consumed it
                        tensor.wait_ge(semM16, g - 1)
                    nc.tensor.matmul(bc_ps[g % 2][:, :npair], ones_row[:],
                                     maskhw[g % 2][:, :npair],
                                     start=True, stop=True
                                     ).then_inc(semBC, 1)

                for g in range(NG):
                    act_mm(g)
                    if g >= 1:
                        bc_mm(g - 1)
                bc_mm(NG - 1)

            @block.gpsimd
            def _(gpsimd):
                # Teardown only.  semY==16 is posted by DVE's last op,
                # which sits after every DVE wait -- transitively it proves
                # EVERY sem reached its final value.  semSYD/semSCD prove
                # sync and scalar issued their last triggers, i.e. passed
                # all their waits.  Store completions post +0: nothing to
                # wait for or clear.  Then zero the sems for the next NEFF
                # execution; the epilogue sweep's ==0 waits gate on this.
                gpsimd.wait_ge(semY, 2 * BS)
                gpsimd.wait_ge(semSYD, 1)
                gpsimd.wait_ge(semSCD, 1)
                if tail == "fast":
                    for rng in compact_to_ranges(sorted(s_.num
                                                        for s_ in all_sems)):
                        gpsimd.dma_reset(rng)
                        gpsimd.sem_clear(rng)

        if tail == "barrier":
            # race-detector-approved: Block exit emitted drains + an
            # all-engine barrier; clear after it.
            for rng in compact_to_ranges(sorted(s_.num for s_ in all_sems)):
                nc.gpsimd.dma_reset(rng)
                nc.gpsimd.sem_clear(rng)

    nc.compile()
    return nc


def get_nc():
    if "nc" not in _cache:
        _cache["nc"] = _build_nc()
    return _cache["nc"]


def kernel(x):
    from concourse.bass_utils import run_bass_kernel_spmd

    x = np.ascontiguousarray(np.asarray(x, dtype=np.float16))
    assert x.shape == (B, C, H, W), x.shape
    nc = get_nc()
    in_maps = [{"x": x[i * BS:(i + 1) * BS]} for i in range(N_CORES)]
    res = run_bass_kernel_spmd(nc, in_maps, list(range(N_CORES)))
    return np.concatenate(
        [res.results[i]["out"] for i in range(N_CORES)], axis=0
    ).astype(np.float32)


# revision 32
# speedup vs baseline: 1.0194x; 1.0018x over previous
"""Trainium2 raw-Bass kernel for nn_BatchDropTop (topk row masking).

Reference math: per sample b, act = sum_c x[b,c,:,:]^2 -> [H,W]; L2-normalize
over flattened (H,W) (positive per-sample scale -- order-preserving, skipped);
row score = max_w act -> [H]; zero the rh=8 rows with the largest score;
out = x * row_mask.

fp16 I/O (host casts): rel-err gate is 2e-2; selection was validated safe with
fp16 inputs + fp32 squares + fp32 accumulation (>=5.4e-6 relative margin on
all 64 samples).  fp16 squares are NOT safe; xsq stays fp32.

RAW Bass (no TileContext), manual semaphores.  Trace-driven structure:

  * The NEFF epilogue (walrus-emitted) makes EVERY engine (a) join a
    sem-2 arrival barrier and then (b) serially wait for every semaphore
    in its fixed ~51-sem hardware window to be 0 (Tensor's chain alone is
    ~55 x 115ns = 6.3us; the sweep also gates on the DMA-bookkeeping sems,
    i.e. it drains the queues).  Consequences engineered for here:
      - barrier-less block end (each engine branches to the end bb) so
        early-finishing engines reach the arrival barrier immediately;
      - all bass sems sit in SYNC's sweep window (207-255, the fastest
        chain at ~23ns/wait);
      - store-completion updates are +0 (nothing to wait on or clear), so
        the teardown does not wait out the final stores' wire time -- the
        epilogue sweep covers it;
      - sync/scalar post a 1-cycle "done" inc after their last trigger so
        the teardown can prove they passed their waits;
      - gpsimd (nothing else to do) waits semY==16 (DVE's last update --
        transitively the final value of EVERY sem), semSYD, semSCD, then
        dma_reset + sem_clear of the one contiguous sem range.
  * gpsimd runs NOTHING in the pipeline (its partition_broadcast measured
    3.7us/sample here vs 0.9us under Tile -- DMA-engine contention).  The
    mask broadcast maskhw[1,·] -> [P,·] is a PE ones-matmul (K=1, fp16
    single-pass) into PSUM, converted fp32->fp16 PSUM->SBUF by ACT
    (ScalarE sits closest to PSUM; ACT has ~2.5us/sample of slack).
  * Samples are processed in GROUPS [(0,),(1,),(2,3),(4,5),(6,7)]: the
    fill samples run alone for latency, later pairs share one DVE
    instruction per stage (fp32 fold fixed cost ~150ns/op plus each
    standalone sem-wait ~170ns of DVE queue time -- pairing halves both).

Dataflow per core (8 samples; per sample x is [P=128, KC=16, HW=192] f16,
partition p holds channels 16p..16p+15):
  loads:   s0 in fold-pair-aligned quarters (q0,q1 ring A / q2,q3 ring B
           so ACT can chase them), s1..s7 full tile on ring A (sync).
           Every load has a DEDICATED completion sem -- no cross-queue
           ordering assumptions.
  ACT:     square f16 -> f32 (one ACTIVATE per sample), m16 PSUM->SBUF
           copies per group, ring B triggers.
  DVE:     (pacer) L1/L2 contiguous fp32 folds per group; rowmax (PSUM),
           MAX8 top8 (per sample), maskhw compare per group; y = x*m16 IN
           PLACE on the x tile in two halves per sample (fp16 2x mode --
           a full-sample multiply loses it).  Software pipelined with
           stage skew: fold[g] | rowmax/max8/mask[g-1] | mults[g-2].
  PE:      four accumulating N=192 fp32 ones-matmuls per sample -> act
           [1, npair*192] PSUM + one mask-broadcast matmul per group.
  stores:  straight from the x tile (in-place mult => no y tiles/WAR).
           s0..s6 full on ring A; s7 in halves on ring B to parallelize
           the end drain across both rings.

The race model does not credit same-engine program order for data
visibility (and HW agrees: removing the same-engine waits broke real-HW
results) -- semDVE is the DVE self-clock; release points inc it, a wait at
value k implies everything program-order-before the k-th release.

Measured facts carried over (do not regress):
  - DVE fp32 tensor_tensor 1x ((N+151)/0.96ns); fp16 TT 2x_1P; strided
    tensor_reduce ~3x slower than contiguous TT folds.
  - fp16 anywhere in the fold tree flips the selection on this input set.
"""

import sys

import numpy as np

for _p in ("/opt/trn_rl_repo", "/root/.axon_site/_ro/trn_rl_repo"):
    if _p not in sys.path:
        sys.path.append(_p)

B, C, H, W = 64, 2048, 24, 8
N_CORES = 8
BS = B // N_CORES  # samples per core
P = 128            # SBUF partitions
KC = C // P        # channel chunks per sample (16)
KH = KC // 2       # 8
KQ = KC // 4       # 4
HW = H * W         # 192
RH = 8             # rows to drop == round(0.33 * 24)

# Sample groups: fill samples alone (pipeline latency), later pairs share
# DVE instructions.
GROUPS = [(0,), (1,), (2, 3), (4, 5), (6, 7)]
NG = len(GROUPS)

# First sem number for this kernel's sems: inside SYNC's NEFF-epilogue
# sweep window (207-255) -- see module docstring.
SEM_BASE = 210

_cache = {}


def _build_nc(tail="fast"):
    """tail="fast": barrier-less block end + minimal teardown (production).
    tail="barrier": standard Block exit (drains + all-engine barrier) +
    post-block clears -- structurally what the CoreSim race detector fully
    validates; the pipeline emission is IDENTICAL, so validating it
    validates the pipeline."""
    from contextlib import ExitStack, contextmanager

    from concourse import bacc, bass, mybir
    from concourse.bass import compact_to_ranges

    f32 = mybir.dt.float32
    f16 = mybir.dt.float16
    ADD = mybir.AluOpType.add
    MULT = mybir.AluOpType.mult

    class _NoBarrierBlock(bass.BassBlock):
        """BassBlock whose exit wires the end bb but emits NO all-engine
        barrier: each engine falls straight into the NEFF epilogue's own
        arrival barrier instead of idling behind a bass one too."""

        def __exit__(self, exc_type, exc_val, exc_tb):
            if exc_type is not None:
                return
            for engine, last_body in self.last_body.items():
                with self.bass.body(
                    last_body, parent=self.bass.cur_bb,
                    allow_existing_parent=True,
                ):
                    engine.br(self.end_bb)
            self.bass.switch_bb(self.end_bb)

    @contextmanager
    def no_barrier_block(nc, name):
        assert nc.cur_block is None
        with _NoBarrierBlock(nc, name) as blk:
            nc.cur_block = blk
            yield blk
        nc.cur_block = None

    nc = bacc.Bacc("TRN2", target_bir_lowering=False, debug=False,
                   num_devices=N_CORES,
                   detect_race_conditions=(tail == "barrier"))
    x_in = nc.dram_tensor("x", [BS, C, H, W], f16, kind="ExternalInput")
    y_out = nc.dram_tensor("out", [BS, C, H, W], f16, kind="ExternalOutput")

    es = ExitStack()
    with es:
        # --- SBUF / PSUM (double-buffered per GROUP) -----------------------
        xt = [es.enter_context(nc.sbuf_tensor(f"x{s}", [P, KC, HW], f16))
              for s in range(BS)]
        xsq = [es.enter_context(
                   nc.sbuf_tensor(f"xsq{i}", [P, 2, KC, HW], f32))
               for i in range(2)]
        t1 = [es.enter_context(nc.sbuf_tensor(f"t1_{i}", [P, 2, KH, HW],
                                              f32)) for i in range(2)]
        t2 = [es.enter_context(nc.sbuf_tensor(f"t2_{i}", [P, 2, KQ, HW],
                                              f32)) for i in range(2)]
        ones = es.enter_context(nc.sbuf_tensor("ones", [P, 1], f32))
        # fp16 so the K=1 broadcast matmul (fp16 x fp16 -> fp32 PSUM) is
        # single-pass; exact for 0/1 mask values.
        ones_row = es.enter_context(nc.sbuf_tensor("ones_row", [1, P], f16))
        rowmax = [es.enter_context(nc.sbuf_tensor(f"rm{i}", [1, 2, H], f32))
                  for i in range(2)]
        top8 = [es.enter_context(nc.sbuf_tensor(f"t8_{i}", [1, 2, RH], f32))
                for i in range(2)]
        maskhw = [es.enter_context(nc.sbuf_tensor(f"mh{i}", [1, 2, HW],
                                                  f16)) for i in range(2)]
        m16 = [es.enter_context(nc.sbuf_tensor(f"m16_{i}", [P, 2, HW], f16))
               for i in range(2)]
        act_ps = [es.enter_context(nc.psum_tensor(f"act{i}", [1, 2, HW],
                                                  f32)) for i in range(2)]
        bc_ps = [es.enter_context(nc.psum_tensor(f"bc{i}", [P, 2, HW], f32))
                 for i in range(2)]

        # --- semaphores (one contiguous range in SYNC's sweep window) ------
        semno = iter(range(SEM_BASE, 256))

        def sem(name):
            return es.enter_context(nc.semaphore(name, num=next(semno)))

        lq = [sem(f"lq{i}") for i in range(4)]      # s0 quarter loads
        lf = {s: sem(f"lf{s}") for s in range(1, BS)}  # full loads
        semSQ = sem("semSQ")      # ACT squares (4 for s0 quarters, 1/sample)
        semT2 = sem("semT2")      # DVE L2 done, 1/GROUP
        semACT = sem("semACT")    # PE act matmul group done, 1/SAMPLE
        semMH = sem("semMH")      # DVE maskhw done, 1/GROUP
        semBC = sem("semBC")      # PE mask-broadcast matmul done, 1/GROUP
        semM16 = sem("semM16")    # ACT m16 copy done, 1/GROUP
        semY = sem("semY")        # DVE mult halves, 2/SAMPLE
        semSTA = sem("semSTA")    # store completions (+0 updates)
        semONES = sem("semONES")  # ones memsets done
        semDVE = sem("semDVE")    # DVE self-clock
        semSYD = sem("semSYD")    # sync issued all triggers (passed waits)
        semSCD = sem("semSCD")    # scalar issued all triggers
        all_sems = (lq + list(lf.values())
                    + [semSQ, semT2, semACT, semMH, semBC, semM16, semY,
                       semSTA, semONES, semDVE, semSYD, semSCD])

        x_dram = [x_in[s].rearrange("(p k) h w -> p k (h w)", p=P)
                  for s in range(BS)]
        y_dram = [y_out[s].rearrange("(p k) h w -> p k (h w)", p=P)
                  for s in range(BS)]

        # semSQ value once ACT's square(s) for sample s are done
        # (s0 = 4 quarters; s1 is squared by the DVE itself in its fill
        # gap, so ACT skips it).
        def sq_val(s):
            assert s != 1
            return 4 if s == 0 else 3 + s

        # DVE clock bookkeeping: dve_clk[tag] = semDVE value after the
        # tagged release op.
        dve_clk = {"n": 0}

        def rel(inst, tag):
            inst.then_inc(semDVE, 1)
            dve_clk["n"] += 1
            dve_clk[tag] = dve_clk["n"]

        # Same-engine DVE data visibility is NOT given by program order
        # (measured on HW: removing these waits broke the results).  One
        # wait per true same-engine edge; acquired knowledge propagates
        # forward in program order.
        def dve_self_wait(vector, val):
            vector.wait_ge(semDVE, val)

        if tail == "fast":
            block_ctx = no_barrier_block(nc, "bdt")
        else:
            block_ctx = nc.Block("bdt", no_gpsimd_drain=True)
        with block_ctx as block:

            @block.sync
            def _(sync):
                # loads first (no deps): s0 quarters q0,q1 then s1..s7 full
                sync.dma_start(out=xt[0][:, 0 * KQ:1 * KQ, :],
                               in_=x_dram[0][:, 0 * KQ:1 * KQ, :]
                               ).then_inc(lq[0], 16)
                sync.dma_start(out=xt[0][:, 1 * KQ:2 * KQ, :],
                               in_=x_dram[0][:, 1 * KQ:2 * KQ, :]
                               ).then_inc(lq[1], 16)
                for s in range(1, BS):
                    sync.dma_start(out=xt[s][:], in_=x_dram[s][:]
                                   ).then_inc(lf[s], 16)
                # stores: x tiles hold y after the in-place multiply.  +0
                # completion updates (see module docstring).  s5/s7 go on
                # ring B (scalar) so the end drain runs on both rings;
                # s6/s7 store in halves to start their wire earlier.
                for s in (0, 1, 2, 3, 4):
                    sync.wait_ge(semY, 2 * s + 2)
                    sync.dma_start(out=y_dram[s][:], in_=xt[s][:]
                                   ).then_inc(semSTA, 0, skip_validation=True)
                s = 6
                for half in range(2):
                    ksl = slice(half * KH, (half + 1) * KH)
                    sync.wait_ge(semY, 2 * s + 1 + half)
                    sync.dma_start(out=y_dram[s][:, ksl, :],
                                   in_=xt[s][:, ksl, :]
                                   ).then_inc(semSTA, 0, skip_validation=True)
                sync.sem_inc(semSYD, 1)

            @block.scalar
            def _(scalar):
                # ring B load triggers up-front: s0 quarters q2, q3.
                scalar.dma_start(out=xt[0][:, 2 * KQ:3 * KQ, :],
                                 in_=x_dram[0][:, 2 * KQ:3 * KQ, :]
                                 ).then_inc(lq[2], 16)
                scalar.dma_start(out=xt[0][:, 3 * KQ:4 * KQ, :],
                                 in_=x_dram[0][:, 3 * KQ:4 * KQ, :]
                                 ).then_inc(lq[3], 16)
                # sample 0 squared quarter-by-quarter in fold-pair order
                # (q0, q2 feed L1 piece A; q1, q3 feed piece B).
                for q in (0, 2, 1, 3):
                    scalar.wait_ge(lq[q], 16)
                    qs = slice(q * KQ, (q + 1) * KQ)
                    nc.scalar.square(xsq[0][:, 0, qs, :], xt[0][:, qs, :]
                                     ).then_inc(semSQ, 1)

                def sq_stage(g, r, s):
                    # xsq buffer WAR: DVE L2 of group g-2 consumed it.
                    scalar.wait_ge(lf[s], 16)
                    if g >= 2:
                        scalar.wait_ge(semT2, g - 1)
                    nc.scalar.square(xsq[g % 2][:, r], xt[s][:]
                                     ).then_inc(semSQ, 1)

                def cp_stage(g):
                    # m16 = fp16(bc_ps[g]): ScalarE is closest to PSUM.
                    # m16 buffer WAR: DVE mults of group g-2 done with it.
                    scalar.wait_ge(semBC, g + 1)
                    if g >= 2:
                        s_hi = GROUPS[g - 2][-1]
                        scalar.wait_ge(semY, 2 * s_hi + 2)
                    npair = len(GROUPS[g])
                    nc.scalar.copy(m16[g % 2][:, :npair],
                                   bc_ps[g % 2][:, :npair]
                                   ).then_inc(semM16, 1)

                # squares chase the loads; each group's copy is emitted
                # two groups behind (its bc matmul needs that group's
                # maskhw, which the DVE produces with one-slot skew).
                done_cp = 0
                for g, grp in enumerate(GROUPS):
                    for r, s in enumerate(grp):
                        if s not in (0, 1):
                            sq_stage(g, r, s)
                    if g >= 2:
                        cp_stage(done_cp)
                        done_cp += 1
                while done_cp < NG:
                    cp_stage(done_cp)
                    done_cp += 1

                # ring B end-drain: s5 full, then s7 in halves.
                s = 5
                scalar.wait_ge(semY, 2 * s + 2)
                scalar.dma_start(out=y_dram[s][:], in_=xt[s][:]
                                 ).then_inc(semSTA, 0, skip_validation=True)
                s = BS - 1
                for half in range(2):
                    ksl = slice(half * KH, (half + 1) * KH)
                    scalar.wait_ge(semY, 2 * s + 1 + half)
                    scalar.dma_start(out=y_dram[s][:, ksl, :],
                                     in_=xt[s][:, ksl, :]
                                     ).then_inc(semSTA, 0,
                                                skip_validation=True)
                scalar.sem_inc(semSCD, 1)

            @block.vector
            def _(vector):
                nc.vector.memset(ones[:], 1.0)
                nc.vector.memset(ones_row[:], 1.0).then_inc(semONES, 1)

                def l_stage(g):
                    grp = GROUPS[g]
                    npair = len(grp)
                    xq, tt1, tt2 = xsq[g % 2], t1[g % 2], t2[g % 2]
                    # t1 buffer WAR vs L2[g-2] read: L1[g-1] released after
                    # L2[g-2] in program order.
                    if g >= 2:
                        dve_self_wait(vector, dve_clk[f"L1_{g - 1}"])
                    if g == 0:
                        # chase the quarter squares (q0+q2 then q1+q3)
                        vector.wait_ge(semSQ, 2)
                        nc.vector.tensor_tensor(
                            tt1[:, 0, 0:KQ, :], xq[:, 0, 0:KQ, :],
                            xq[:, 0, 2 * KQ:3 * KQ, :], op=ADD)
                        vector.wait_ge(semSQ, 4)
                        rel(nc.vector.tensor_tensor(
                            tt1[:, 0, KQ:, :], xq[:, 0, KQ:2 * KQ, :],
                            xq[:, 0, 3 * KQ:, :], op=ADD), f"L1_{g}")
                    elif g == 1:
                        dve_self_wait(vector, dve_clk["SQ1"])
                        rel(nc.vector.tensor_tensor(
                            tt1[:, :npair], xq[:, :npair, :KH, :],
                            xq[:, :npair, KH:, :], op=ADD), f"L1_{g}")
                    else:
                        vector.wait_ge(semSQ, sq_val(grp[-1]))
                        rel(nc.vector.tensor_tensor(
                            tt1[:, :npair], xq[:, :npair, :KH, :],
                            xq[:, :npair, KH:, :], op=ADD), f"L1_{g}")
                    # L2 fold; t2 buffer WAR: PE done with group g-2
                    if g >= 2:
                        vector.wait_ge(semACT, GROUPS[g - 2][-1] + 1)
                    dve_self_wait(vector, dve_clk[f"L1_{g}"])
                    nc.vector.tensor_tensor(
                        tt2[:, :npair], tt1[:, :npair, :KQ, :],
                        tt1[:, :npair, KQ:, :], op=ADD
                    ).then_inc(semT2, 1)

                def r_stage(g):
                    grp = GROUPS[g]
                    npair = len(grp)
                    rm, t8, mh = rowmax[g % 2], top8[g % 2], maskhw[g % 2]
                    vector.wait_ge(semACT, grp[-1] + 1)
                    # rm/t8 buffer WAR vs maskhw[g-2] reads: rowmax[g-1]
                    # released after maskhw[g-2] in program order.
                    if g >= 2:
                        dve_self_wait(vector, dve_clk[f"RM_{g - 1}"])
                    rel(nc.vector.tensor_reduce(
                        rm[:, :npair],
                        act_ps[g % 2][:, :npair].rearrange(
                            "p n (h w) -> p n h w", h=H),
                        axis=mybir.AxisListType.X,
                        op=mybir.AluOpType.max), f"RM_{g}")
                    dve_self_wait(vector, dve_clk[f"RM_{g}"])
                    for r in range(npair):
                        rel(nc.vector.max(t8[:, r], rm[:, r]), f"M8_{g}")
                    # maskhw buffer WAR: PE bcast of g-2 done reading it
                    if g >= 2:
                        vector.wait_ge(semBC, g - 1)
                    dve_self_wait(vector, dve_clk[f"M8_{g}"])
                    # mask = (rowmax < per-sample 8th-largest), fp16 0/1
                    nc.vector.tensor_tensor(
                        mh[:, :npair].rearrange("p n (h w) -> p n h w",
                                                h=H),
                        rm[:, :npair].unsqueeze(3).broadcast_to(
                            [1, npair, H, W]),
                        t8[:, :npair, RH - 1:RH].broadcast_to(
                            [1, npair, H]).unsqueeze(3).broadcast_to(
                            [1, npair, H, W]),
                        op=mybir.AluOpType.is_lt,
                    ).then_inc(semMH, 1)

                def m_stage(g):
                    # y = x * m16 in place, two halves per sample (fp16 2x
                    # mode).  Upstream deps arrive transitively through
                    # semM16's acquire chain.
                    vector.wait_ge(semM16, g + 1)
                    for r, s in enumerate(GROUPS[g]):
                        mb = m16[g % 2][:, r].unsqueeze(1).broadcast_to(
                            [P, KH, HW])
                        for half in range(2):
                            ksl = slice(half * KH, (half + 1) * KH)
                            nc.vector.tensor_tensor(
                                xt[s][:, ksl, :], xt[s][:, ksl, :], mb,
                                op=MULT).then_inc(semY, 1)

                for slot in range(NG + 2):
                    if slot == 1:
                        # square s1 on the DVE itself: during the fill the
                        # DVE would otherwise idle waiting for ACT, which
                        # is the serial bottleneck early on.
                        vector.wait_ge(lf[1], 16)
                        rel(nc.vector.tensor_tensor(
                            xsq[1][:, 0], xt[1][:], xt[1][:], op=MULT),
                            "SQ1")
                    if slot < NG:
                        l_stage(slot)
                    if 1 <= slot <= NG:
                        r_stage(slot - 1)
                    if slot >= 2:
                        m_stage(slot - 2)

            @block.tensor
            def _(tensor):
                tensor.wait_ge(semONES, 1)

                def act_mm(g):
                    grp = GROUPS[g]
                    tensor.wait_ge(semT2, g + 1)
                    if g >= 2:
                        # act_ps WAR: DVE rowmax of g-2 consumed it
                        tensor.wait_ge(semDVE, dve_clk[f"RM_{g - 2}"])
                    for r, s in enumerate(grp):
                        for j in range(KQ):
                            mm = nc.tensor.matmul(
                                act_ps[g % 2][:, r], ones[:],
                                t2[g % 2][:, r, j, :],
                                start=(j == 0), stop=(j == KQ - 1))
                        mm.then_inc(semACT, 1)

                def bc_mm(g):
                    # broadcast maskhw[1,npair*HW] to all partitions:
                    # K=1 fp16 matmul -> bc_ps [P, npair*HW] fp32.
                    npair = len(GROUPS[g])
                    tensor.wait_ge(semMH, g + 1)
                    if g >= 2:
                        # bc_ps WAR: ACT copy of g-2 consumed it
                        tensor.wait_ge(semM16, g - 1)
                    nc.tensor.matmul(bc_ps[g % 2][:, :npair], ones_row[:],
                                     maskhw[g % 2][:, :npair],
                                     start=True, stop=True
                                     ).then_inc(semBC, 1)

                for g in range(NG):
                    act_mm(g)
                    if g >= 1:
                        bc_mm(g - 1)
                bc_mm(NG - 1)

            @block.gpsimd
            def _(gpsimd):
                # Teardown only.  semY==16 is posted by DVE's last op,
                # which sits after every DVE wait -- transitively it proves
                # EVERY sem reached its final value.  semSYD/semSCD prove
                # sync and scalar issued their last triggers, i.e. passed
                # all their waits.  Store completions post +0: nothing to
                # wait for or clear.  Then zero the sems for the next NEFF
                # execution; the epilogue sweep's ==0 waits gate on this.
                gpsimd.wait_ge(semY, 2 * BS)
                gpsimd.wait_ge(semSYD, 1)
                gpsimd.wait_ge(semSCD, 1)
                if tail == "fast":
                    for rng in compact_to_ranges(sorted(s_.num
                                                        for s_ in all_sems)):
                        gpsimd.dma_reset(rng)
                        gpsimd.sem_clear(rng)

        if tail == "barrier":
            # race-detector-approved: Block exit emitted drains + an
            # all-engine barrier; clear after it.
            for rng in compact_to_ranges(sorted(s_.num for s_ in all_sems)):
                nc.gpsimd.dma_reset(rng)
                nc.gpsimd.sem_clear(rng)

    nc.compile()
    return nc


def get_nc():
    if "nc" not in _cache:
        _cache["nc"] = _build_nc()
    return _cache["nc"]


def kernel(x):
    from concourse.bass_utils import run_bass_kernel_spmd

    x = np.ascontiguousarray(np.asarray(x, dtype=np.float16))
    assert x.shape == (B, C, H, W), x.shape
    nc = get_nc()
    in_maps = [{"x": x[i * BS:(i + 1) * BS]} for i in range(N_CORES)]
    res = run_bass_kernel_spmd(nc, in_maps, list(range(N_CORES)))
    return np.concatenate(
        [res.results[i]["out"] for i in range(N_CORES)], axis=0
    ).astype(np.float32)


# revision 33
# speedup vs baseline: 1.0223x; 1.0028x over previous
"""Trainium2 raw-Bass kernel for nn_BatchDropTop (topk row masking).

Reference math: per sample b, act = sum_c x[b,c,:,:]^2 -> [H,W]; L2-normalize
over flattened (H,W) (positive per-sample scale -- order-preserving, skipped);
row score = max_w act -> [H]; zero the rh=8 rows with the largest score;
out = x * row_mask.

fp16 I/O (host casts): rel-err gate is 2e-2; selection was validated safe with
fp16 inputs + fp32 squares + fp32 accumulation (>=5.4e-6 relative margin on
all 64 samples).  fp16 squares are NOT safe; xsq stays fp32.

RAW Bass (no TileContext), manual semaphores.  Trace-driven structure:

  * The NEFF epilogue (walrus-emitted) makes EVERY engine (a) join a
    sem-2 arrival barrier and then (b) serially wait for every semaphore
    in its fixed ~51-sem hardware window to be 0 (Tensor's chain alone is
    ~55 x 115ns = 6.3us; the sweep also gates on the DMA-bookkeeping sems,
    i.e. it drains the queues).  Consequences engineered for here:
      - barrier-less block end (each engine branches to the end bb) so
        early-finishing engines reach the arrival barrier immediately;
      - all bass sems sit in SYNC's sweep window (207-255, the fastest
        chain at ~23ns/wait);
      - store-completion updates are +0 (nothing to wait on or clear), so
        the teardown does not wait out the final stores' wire time -- the
        epilogue sweep covers it;
      - sync/scalar post a 1-cycle "done" inc after their last trigger so
        the teardown can prove they passed their waits;
      - gpsimd (nothing else to do) waits semY==16 (DVE's last update --
        transitively the final value of EVERY sem), semSYD, semSCD, then
        dma_reset + sem_clear of the one contiguous sem range.
  * gpsimd runs NOTHING in the pipeline (its partition_broadcast measured
    3.7us/sample here vs 0.9us under Tile -- DMA-engine contention).  The
    mask broadcast maskhw[1,·] -> [P,·] is a PE ones-matmul (K=1, fp16
    single-pass) into PSUM, converted fp32->fp16 PSUM->SBUF by ACT
    (ScalarE sits closest to PSUM; ACT has ~2.5us/sample of slack).
  * Samples are processed in GROUPS [(0,),(1,),(2,3),(4,5),(6,7)]: the
    fill samples run alone for latency, later pairs share one DVE
    instruction per stage (fp32 fold fixed cost ~150ns/op plus each
    standalone sem-wait ~170ns of DVE queue time -- pairing halves both).

Dataflow per core (8 samples; per sample x is [P=128, KC=16, HW=192] f16,
partition p holds channels 16p..16p+15):
  loads:   s0 in fold-pair-aligned quarters (q0,q1 ring A / q2,q3 ring B
           so ACT can chase them), s1..s7 full tile on ring A (sync).
           Every load has a DEDICATED completion sem -- no cross-queue
           ordering assumptions.
  ACT:     square f16 -> f32 (one ACTIVATE per sample), m16 PSUM->SBUF
           copies per group, ring B triggers.
  DVE:     (pacer) L1/L2 contiguous fp32 folds per group; rowmax (PSUM),
           MAX8 top8 (per sample), maskhw compare per group; y = x*m16 IN
           PLACE on the x tile in two halves per sample (fp16 2x mode --
           a full-sample multiply loses it).  Software pipelined with
           stage skew: fold[g] | rowmax/max8/mask[g-1] | mults[g-2].
  PE:      four accumulating N=192 fp32 ones-matmuls per sample -> act
           [1, npair*192] PSUM + one mask-broadcast matmul per group.
  stores:  straight from the x tile (in-place mult => no y tiles/WAR).
           s0..s6 full on ring A; s7 in halves on ring B to parallelize
           the end drain across both rings.

The race model does not credit same-engine program order for data
visibility (and HW agrees: removing the same-engine waits broke real-HW
results) -- semDVE is the DVE self-clock; release points inc it, a wait at
value k implies everything program-order-before the k-th release.

Measured facts carried over (do not regress):
  - DVE fp32 tensor_tensor 1x ((N+151)/0.96ns); fp16 TT 2x_1P; strided
    tensor_reduce ~3x slower than contiguous TT folds.
  - fp16 anywhere in the fold tree flips the selection on this input set.
"""

import sys

import numpy as np

for _p in ("/opt/trn_rl_repo", "/root/.axon_site/_ro/trn_rl_repo"):
    if _p not in sys.path:
        sys.path.append(_p)

B, C, H, W = 64, 2048, 24, 8
N_CORES = 8
BS = B // N_CORES  # samples per core
P = 128            # SBUF partitions
KC = C // P        # channel chunks per sample (16)
KH = KC // 2       # 8
KQ = KC // 4       # 4
HW = H * W         # 192
RH = 8             # rows to drop == round(0.33 * 24)

# Sample groups: fill samples alone (pipeline latency), later pairs share
# DVE instructions.
GROUPS = [(0,), (1,), (2, 3), (4, 5), (6, 7)]
NG = len(GROUPS)

# First sem number for this kernel's sems: inside SYNC's NEFF-epilogue
# sweep window (207-255) -- see module docstring.
SEM_BASE = 210

_cache = {}


def _build_nc(tail="fast"):
    """tail="fast": barrier-less block end + minimal teardown (production).
    tail="barrier": standard Block exit (drains + all-engine barrier) +
    post-block clears -- structurally what the CoreSim race detector fully
    validates; the pipeline emission is IDENTICAL, so validating it
    validates the pipeline."""
    from contextlib import ExitStack, contextmanager

    from concourse import bacc, bass, mybir
    from concourse.bass import compact_to_ranges

    f32 = mybir.dt.float32
    f16 = mybir.dt.float16
    ADD = mybir.AluOpType.add
    MULT = mybir.AluOpType.mult

    class _NoBarrierBlock(bass.BassBlock):
        """BassBlock whose exit wires the end bb but emits NO all-engine
        barrier: each engine falls straight into the NEFF epilogue's own
        arrival barrier instead of idling behind a bass one too."""

        def __exit__(self, exc_type, exc_val, exc_tb):
            if exc_type is not None:
                return
            for engine, last_body in self.last_body.items():
                with self.bass.body(
                    last_body, parent=self.bass.cur_bb,
                    allow_existing_parent=True,
                ):
                    engine.br(self.end_bb)
            self.bass.switch_bb(self.end_bb)

    @contextmanager
    def no_barrier_block(nc, name):
        assert nc.cur_block is None
        with _NoBarrierBlock(nc, name) as blk:
            nc.cur_block = blk
            yield blk
        nc.cur_block = None

    nc = bacc.Bacc("TRN2", target_bir_lowering=False, debug=False,
                   num_devices=N_CORES,
                   detect_race_conditions=(tail == "barrier"))
    x_in = nc.dram_tensor("x", [BS, C, H, W], f16, kind="ExternalInput")
    y_out = nc.dram_tensor("out", [BS, C, H, W], f16, kind="ExternalOutput")

    es = ExitStack()
    with es:
        # --- SBUF / PSUM (double-buffered per GROUP) -----------------------
        xt = [es.enter_context(nc.sbuf_tensor(f"x{s}", [P, KC, HW], f16))
              for s in range(BS)]
        xsq = [es.enter_context(
                   nc.sbuf_tensor(f"xsq{i}", [P, 2, KC, HW], f32))
               for i in range(2)]
        t1 = [es.enter_context(nc.sbuf_tensor(f"t1_{i}", [P, 2, KH, HW],
                                              f32)) for i in range(2)]
        t2 = [es.enter_context(nc.sbuf_tensor(f"t2_{i}", [P, 2, KQ, HW],
                                              f32)) for i in range(2)]
        ones = es.enter_context(nc.sbuf_tensor("ones", [P, 1], f32))
        # fp16 so the K=1 broadcast matmul (fp16 x fp16 -> fp32 PSUM) is
        # single-pass; exact for 0/1 mask values.
        ones_row = es.enter_context(nc.sbuf_tensor("ones_row", [1, P], f16))
        rowmax = [es.enter_context(nc.sbuf_tensor(f"rm{i}", [1, 2, H], f32))
                  for i in range(2)]
        top8 = [es.enter_context(nc.sbuf_tensor(f"t8_{i}", [1, 2, RH], f32))
                for i in range(2)]
        maskhw = [es.enter_context(nc.sbuf_tensor(f"mh{i}", [1, 2, HW],
                                                  f16)) for i in range(2)]
        m16 = [es.enter_context(nc.sbuf_tensor(f"m16_{i}", [P, 2, HW], f16))
               for i in range(2)]
        act_ps = [es.enter_context(nc.psum_tensor(f"act{i}", [1, 2, HW],
                                                  f32)) for i in range(2)]
        bc_ps = [es.enter_context(nc.psum_tensor(f"bc{i}", [P, 2, HW], f32))
                 for i in range(2)]

        # --- semaphores (one contiguous range in SYNC's sweep window) ------
        semno = iter(range(SEM_BASE, 256))

        def sem(name):
            return es.enter_context(nc.semaphore(name, num=next(semno)))

        lq = [sem(f"lq{i}") for i in range(2)]      # s0 fold-half loads
        lf = {s: sem(f"lf{s}") for s in range(1, BS)}  # full loads
        semSQ = sem("semSQ")      # ACT squares (4 for s0 quarters, 1/sample)
        semT2 = sem("semT2")      # DVE L2 done, 1/GROUP
        semACT = sem("semACT")    # PE act matmul group done, 1/SAMPLE
        semMH = sem("semMH")      # DVE maskhw done, 1/GROUP
        semBC = sem("semBC")      # PE mask-broadcast matmul done, 1/GROUP
        semM16 = sem("semM16")    # ACT m16 copy done, 1/GROUP
        semY = sem("semY")        # DVE mult halves, 2/SAMPLE
        semSTA = sem("semSTA")    # store completions (+0 updates)
        semONES = sem("semONES")  # ones memsets done
        semDVE = sem("semDVE")    # DVE self-clock
        semSYD = sem("semSYD")    # sync issued all triggers (passed waits)
        semSCD = sem("semSCD")    # scalar issued all triggers
        all_sems = (lq + list(lf.values())
                    + [semSQ, semT2, semACT, semMH, semBC, semM16, semY,
                       semSTA, semONES, semDVE, semSYD, semSCD])

        x_dram = [x_in[s].rearrange("(p k) h w -> p k (h w)", p=P)
                  for s in range(BS)]
        y_dram = [y_out[s].rearrange("(p k) h w -> p k (h w)", p=P)
                  for s in range(BS)]

        # semSQ value once ACT's square(s) for sample s are done
        # (s0 = 2 fold-halves; s1 is squared by the DVE itself in its
        # fill gap, so ACT skips it).
        def sq_val(s):
            assert s != 1
            return 2 if s == 0 else 1 + s

        # DVE clock bookkeeping: dve_clk[tag] = semDVE value after the
        # tagged release op.
        dve_clk = {"n": 0}

        def rel(inst, tag):
            inst.then_inc(semDVE, 1)
            dve_clk["n"] += 1
            dve_clk[tag] = dve_clk["n"]

        # Same-engine DVE data visibility is NOT given by program order
        # (measured on HW: removing these waits broke the results).  One
        # wait per true same-engine edge; acquired knowledge propagates
        # forward in program order.
        def dve_self_wait(vector, val):
            vector.wait_ge(semDVE, val)

        if tail == "fast":
            block_ctx = no_barrier_block(nc, "bdt")
        else:
            block_ctx = nc.Block("bdt", no_gpsimd_drain=True)
        with block_ctx as block:

            @block.sync
            def _(sync):
                # loads first (no deps): s0 in two FOLD-HALF transfers
                # (k in {0:4, 8:12} then {4:8, 12:16}) so each delivers
                # exactly what one L1 fold piece consumes; then s1..s7
                # full.  All on ring A: ring B's triggers sit behind the
                # ACT engine's ~7us preamble + table load (measured to
                # delay the fill by ~4us), and fewer s0 triggers get s1's
                # load out earlier.
                x0v = x_in[0].rearrange("(p j k) h w -> p j k (h w)",
                                        p=P, j=2)
                xt0v = xt[0][:].rearrange("p (j k) hw -> p j k hw", j=2)
                for h_ in range(2):
                    ks = slice(h_ * KQ, (h_ + 1) * KQ)
                    sync.dma_start(out=xt0v[:, :, ks, :],
                                   in_=x0v[:, :, ks, :]
                                   ).then_inc(lq[h_], 16)
                for s in range(1, BS):
                    sync.dma_start(out=xt[s][:], in_=x_dram[s][:]
                                   ).then_inc(lf[s], 16)
                # stores: x tiles hold y after the in-place multiply.  +0
                # completion updates (see module docstring).  s5/s7 go on
                # ring B (scalar) so the end drain runs on both rings;
                # s6/s7 store in halves to start their wire earlier.
                for s in (0, 1, 2, 3, 4):
                    sync.wait_ge(semY, 2 * s + 2)
                    sync.dma_start(out=y_dram[s][:], in_=xt[s][:]
                                   ).then_inc(semSTA, 0, skip_validation=True)
                s = 6
                for half in range(2):
                    ksl = slice(half * KH, (half + 1) * KH)
                    sync.wait_ge(semY, 2 * s + 1 + half)
                    sync.dma_start(out=y_dram[s][:, ksl, :],
                                   in_=xt[s][:, ksl, :]
                                   ).then_inc(semSTA, 0, skip_validation=True)
                sync.sem_inc(semSYD, 1)

            @block.scalar
            def _(scalar):
                # sample 0 squared in the two fold-half pieces.
                xt0v = xt[0][:].rearrange("p (j k) hw -> p j k hw", j=2)
                xsq0v = xsq[0][:, 0].rearrange("p (j k) hw -> p j k hw",
                                               j=2)
                for h_ in range(2):
                    ks = slice(h_ * KQ, (h_ + 1) * KQ)
                    scalar.wait_ge(lq[h_], 16)
                    nc.scalar.square(xsq0v[:, :, ks, :], xt0v[:, :, ks, :]
                                     ).then_inc(semSQ, 1)

                def sq_stage(g, r, s):
                    # xsq buffer WAR: DVE L2 of group g-2 consumed it.
                    scalar.wait_ge(lf[s], 16)
                    if g >= 2:
                        scalar.wait_ge(semT2, g - 1)
                    nc.scalar.square(xsq[g % 2][:, r], xt[s][:]
                                     ).then_inc(semSQ, 1)

                def cp_stage(g):
                    # m16 = fp16(bc_ps[g]): ScalarE is closest to PSUM.
                    # m16 buffer WAR: DVE mults of group g-2 done with it.
                    scalar.wait_ge(semBC, g + 1)
                    if g >= 2:
                        s_hi = GROUPS[g - 2][-1]
                        scalar.wait_ge(semY, 2 * s_hi + 2)
                    npair = len(GROUPS[g])
                    nc.scalar.copy(m16[g % 2][:, :npair],
                                   bc_ps[g % 2][:, :npair]
                                   ).then_inc(semM16, 1)

                # squares chase the loads; each group's copy is emitted
                # two groups behind (its bc matmul needs that group's
                # maskhw, which the DVE produces with one-slot skew).
                done_cp = 0
                for g, grp in enumerate(GROUPS):
                    for r, s in enumerate(grp):
                        if s not in (0, 1):
                            sq_stage(g, r, s)
                    if g >= 2:
                        cp_stage(done_cp)
                        done_cp += 1
                while done_cp < NG:
                    cp_stage(done_cp)
                    done_cp += 1

                # ring B end-drain: s5 full, then s7 in halves.
                s = 5
                scalar.wait_ge(semY, 2 * s + 2)
                scalar.dma_start(out=y_dram[s][:], in_=xt[s][:]
                                 ).then_inc(semSTA, 0, skip_validation=True)
                s = BS - 1
                for half in range(2):
                    ksl = slice(half * KH, (half + 1) * KH)
                    scalar.wait_ge(semY, 2 * s + 1 + half)
                    scalar.dma_start(out=y_dram[s][:, ksl, :],
                                     in_=xt[s][:, ksl, :]
                                     ).then_inc(semSTA, 0,
                                                skip_validation=True)
                scalar.sem_inc(semSCD, 1)

            @block.vector
            def _(vector):
                nc.vector.memset(ones[:], 1.0)
                nc.vector.memset(ones_row[:], 1.0).then_inc(semONES, 1)

                def l_stage(g):
                    grp = GROUPS[g]
                    npair = len(grp)
                    xq, tt1, tt2 = xsq[g % 2], t1[g % 2], t2[g % 2]
                    # t1 buffer WAR vs L2[g-2] read: L1[g-1] released after
                    # L2[g-2] in program order.
                    if g >= 2:
                        dve_self_wait(vector, dve_clk[f"L1_{g - 1}"])
                    if g == 0:
                        # chase the fold-half squares
                        vector.wait_ge(semSQ, 1)
                        nc.vector.tensor_tensor(
                            tt1[:, 0, 0:KQ, :], xq[:, 0, 0:KQ, :],
                            xq[:, 0, 2 * KQ:3 * KQ, :], op=ADD)
                        vector.wait_ge(semSQ, 2)
                        rel(nc.vector.tensor_tensor(
                            tt1[:, 0, KQ:, :], xq[:, 0, KQ:2 * KQ, :],
                            xq[:, 0, 3 * KQ:, :], op=ADD), f"L1_{g}")
                    elif g == 1:
                        dve_self_wait(vector, dve_clk["SQ1"])
                        rel(nc.vector.tensor_tensor(
                            tt1[:, :npair], xq[:, :npair, :KH, :],
                            xq[:, :npair, KH:, :], op=ADD), f"L1_{g}")
                    else:
                        vector.wait_ge(semSQ, sq_val(grp[-1]))
                        rel(nc.vector.tensor_tensor(
                            tt1[:, :npair], xq[:, :npair, :KH, :],
                            xq[:, :npair, KH:, :], op=ADD), f"L1_{g}")
                    # L2 fold; t2 buffer WAR: PE done with group g-2
                    if g >= 2:
                        vector.wait_ge(semACT, GROUPS[g - 2][-1] + 1)
                    dve_self_wait(vector, dve_clk[f"L1_{g}"])
                    nc.vector.tensor_tensor(
                        tt2[:, :npair], tt1[:, :npair, :KQ, :],
                        tt1[:, :npair, KQ:, :], op=ADD
                    ).then_inc(semT2, 1)

                def r_stage(g):
                    grp = GROUPS[g]
                    npair = len(grp)
                    rm, t8, mh = rowmax[g % 2], top8[g % 2], maskhw[g % 2]
                    vector.wait_ge(semACT, grp[-1] + 1)
                    # rm/t8 buffer WAR vs maskhw[g-2] reads: rowmax[g-1]
                    # released after maskhw[g-2] in program order.
                    if g >= 2:
                        dve_self_wait(vector, dve_clk[f"RM_{g - 1}"])
                    rel(nc.vector.tensor_reduce(
                        rm[:, :npair],
                        act_ps[g % 2][:, :npair].rearrange(
                            "p n (h w) -> p n h w", h=H),
                        axis=mybir.AxisListType.X,
                        op=mybir.AluOpType.max), f"RM_{g}")
                    dve_self_wait(vector, dve_clk[f"RM_{g}"])
                    for r in range(npair):
                        rel(nc.vector.max(t8[:, r], rm[:, r]), f"M8_{g}")
                    # maskhw buffer WAR: PE bcast of g-2 done reading it
                    if g >= 2:
                        vector.wait_ge(semBC, g - 1)
                    dve_self_wait(vector, dve_clk[f"M8_{g}"])
                    # mask = (rowmax < per-sample 8th-largest), fp16 0/1
                    nc.vector.tensor_tensor(
                        mh[:, :npair].rearrange("p n (h w) -> p n h w",
                                                h=H),
                        rm[:, :npair].unsqueeze(3).broadcast_to(
                            [1, npair, H, W]),
                        t8[:, :npair, RH - 1:RH].broadcast_to(
                            [1, npair, H]).unsqueeze(3).broadcast_to(
                            [1, npair, H, W]),
                        op=mybir.AluOpType.is_lt,
                    ).then_inc(semMH, 1)

                def m_stage(g):
                    # y = x * m16 in place, two halves per sample (fp16 2x
                    # mode).  Upstream deps arrive transitively through
                    # semM16's acquire chain.
                    vector.wait_ge(semM16, g + 1)
                    for r, s in enumerate(GROUPS[g]):
                        mb = m16[g % 2][:, r].unsqueeze(1).broadcast_to(
                            [P, KH, HW])
                        for half in range(2):
                            ksl = slice(half * KH, (half + 1) * KH)
                            nc.vector.tensor_tensor(
                                xt[s][:, ksl, :], xt[s][:, ksl, :], mb,
                                op=MULT).then_inc(semY, 1)

                for slot in range(NG + 2):
                    if slot == 1:
                        # square s1 on the DVE itself: during the fill the
                        # DVE would otherwise idle waiting for ACT, which
                        # is the serial bottleneck early on.
                        vector.wait_ge(lf[1], 16)
                        rel(nc.vector.tensor_tensor(
                            xsq[1][:, 0], xt[1][:], xt[1][:], op=MULT),
                            "SQ1")
                    if slot < NG:
                        l_stage(slot)
                    if 1 <= slot <= NG:
                        r_stage(slot - 1)
                    if slot >= 2:
                        m_stage(slot - 2)

            @block.tensor
            def _(tensor):
                tensor.wait_ge(semONES, 1)

                def act_mm(g):
                    grp = GROUPS[g]
                    tensor.wait_ge(semT2, g + 1)
                    if g >= 2:
                        # act_ps WAR: DVE rowmax of g-2 consumed it
                        tensor.wait_ge(semDVE, dve_clk[f"RM_{g - 2}"])
                    for r, s in enumerate(grp):
                        for j in range(KQ):
                            mm = nc.tensor.matmul(
                                act_ps[g % 2][:, r], ones[:],
                                t2[g % 2][:, r, j, :],
                                start=(j == 0), stop=(j == KQ - 1))
                        mm.then_inc(semACT, 1)

                def bc_mm(g):
                    # broadcast maskhw[1,npair*HW] to all partitions:
                    # K=1 fp16 matmul -> bc_ps [P, npair*HW] fp32.
                    npair = len(GROUPS[g])
                    tensor.wait_ge(semMH, g + 1)
                    if g >= 2:
                        # bc_ps WAR: ACT copy of g-2 consumed it
                        tensor.wait_ge(semM16, g - 1)
                    nc.tensor.matmul(bc_ps[g % 2][:, :npair], ones_row[:],
                                     maskhw[g % 2][:, :npair],
                                     start=True, stop=True
                                     ).then_inc(semBC, 1)

                for g in range(NG):
                    act_mm(g)
                    if g >= 1:
                        bc_mm(g - 1)
                bc_mm(NG - 1)

            @block.gpsimd
            def _(gpsimd):
                # Teardown only.  semY==16 is posted by DVE's last op,
                # which sits after every DVE wait -- transitively it proves
                # EVERY sem reached its final value.  semSYD/semSCD prove
                # sync and scalar issued their last triggers, i.e. passed
                # all their waits.  Store completions post +0: nothing to
                # wait for or clear.  Then zero the sems for the next NEFF
                # execution; the epilogue sweep's ==0 waits gate on this.
                gpsimd.wait_ge(semY, 2 * BS)
                gpsimd.wait_ge(semSYD, 1)
                gpsimd.wait_ge(semSCD, 1)
                if tail == "fast":
                    for rng in compact_to_ranges(sorted(s_.num
                                                        for s_ in all_sems)):
                        gpsimd.dma_reset(rng)
                        gpsimd.sem_clear(rng)

        if tail == "barrier":
            # race-detector-approved: Block exit emitted drains + an
            # all-engine barrier; clear after it.
            for rng in compact_to_ranges(sorted(s_.num for s_ in all_sems)):
                nc.gpsimd.dma_reset(rng)
                nc.gpsimd.sem_clear(rng)

    nc.compile()
    return nc


def get_nc():
    if "nc" not in _cache:
        _cache["nc"] = _build_nc()
    return _cache["nc"]


def kernel(x):
    from concourse.bass_utils import run_bass_kernel_spmd

    x = np.ascontiguousarray(np.asarray(x, dtype=np.float16))
    assert x.shape == (B, C, H, W), x.shape
    nc = get_nc()
    in_maps = [{"x": x[i * BS:(i + 1) * BS]} for i in range(N_CORES)]
    res = run_bass_kernel_spmd(nc, in_maps, list(range(N_CORES)))
    return np.concatenate(
        [res.results[i]["out"] for i in range(N_CORES)], axis=0
    ).astype(np.float32)
